# revision 82
# baseline (speedup 1.0000x reference)
"""LocalMHA (windowed attention) Trainium2 Bass kernel.

Full inputs -> full outputs. Internally: 8-way data-parallel over
(batch, token-half) shards; each NeuronCore runs the complete block on
4096 tokens (32 windows of 128). No collectives.

Problem (hardcoded):
  x: (4, 1024, 8192) f32, DIM=1024, HEADS=16, DIM_HEAD=64, WINDOW=128
  out = W_out @ attn(LN(x)) + x   (per reference.py)
"""

import numpy as np
import ml_dtypes

import concourse.bass as bass
import concourse.bacc as bacc
import concourse.tile as tile
from concourse import mybir
from concourse.bass_utils import run_bass_kernel_spmd

BF16 = mybir.dt.bfloat16
F32 = mybir.dt.float32
F32R = mybir.dt.float32r
FP8 = mybir.dt.float8e4
WSCALE = 16.0       # fp8 weights stored x16; psum drains divide back out
DR = mybir.MatmulPerfMode.DoubleRow

B, DIM, T = 4, 1024, 8192
HEADS, DHEAD, WIN = 16, 64, 128
NCORE = 8
NTOK = (B * T) // NCORE          # 4096 tokens per core
NT = 512                         # token tile
NTILES = NTOK // NT              # 8
KC = DIM // 128                  # 8 contraction chunks
WPT = NT // WIN                  # 4 windows per token tile
EPS = 1e-5

_CACHED = {}
PHASE_LOG = []

# build-time tuning knobs (swept via TimelineSim)
CFG = dict(
    sel_dma=False,      # denom broadcast via DRAM-bounce DMA vs sel matmuls
    psa_bufs=8,
    qk_bufs=2,
    tp_bufs=4,
    ln_at=-1,
    xs_at=3,
    op_batches=8,
    qk_in_attn=[],
    attn_order=[('s',0),('s',1),('s',2),('d',0),('s',3),('d',1),
                ('b',0),('d',2),('b',1),('d',3),('b',2),('b',3)],
    vp_bufs=2,
    ap_bufs=2,
    xp_bufs=2,
)
# engine per x^2 chunk (distributes the serial LN-squares chain)
SQ_ENG = ["act", "pool", "vec", "act", "pool", "vec", "act", "pool"]


def _mark(nc, phase):
    PHASE_LOG.append((phase, len(nc.inst_map)))


def _legalize_waits(nc):
    """This toolchain's walrus encodes at most ONE sem-wait per instruction
    (ISA EVENTS struct has a single wait slot) and errors with 'Too many sync
    wait commands' on Tile's multi-wait output. Split: hoist all but one wait
    onto same-engine ENGINE_NOPs inserted immediately before the instruction
    (engine stalls there first -> identical ordering semantics)."""
    eng_map = {
        mybir.EngineType.PE: nc.tensor,
        mybir.EngineType.Activation: nc.scalar,
        mybir.EngineType.DVE: nc.vector,
        mybir.EngineType.Pool: nc.gpsimd,
        mybir.EngineType.SP: nc.sync,
    }
    for f in nc.m.functions:
        for bb in f.blocks:
            lst = bb.instructions  # live list
            need = [
                i for i in lst
                if i.sync_info is not None and len(i.sync_info.on_wait) > 1
            ]
            for inst in need:
                si = inst.sync_info
                waits = list(si.on_wait)
                nops = []
                for w in waits[:-1]:
                    eng = eng_map[inst.engine]
                    bnop = eng.isa(
                        nc.isa.Opcode.NEURON_ISA_TPB_OPCODE_ENGINE_NOP, {}
                    )
                    ni = bnop.ins
                    # engine_nop appended to the current bb; remove it
                    removed = False
                    for f2 in nc.m.functions:
                        for bb2 in f2.blocks:
                            l2 = bb2.instructions
                            if l2 and l2[-1] is ni:
                                l2.pop()
                                removed = True
                                break
                        if removed:
                            break
                    assert removed, "could not relocate wait nop"
                    ni.sync_info = mybir.SyncInfo(on_wait=[w], on_update=[])
                    nops.append(ni)
                inst.sync_info = mybir.SyncInfo(
                    on_wait=[waits[-1]], on_update=list(si.on_update)
                )
                idx = None
                for j in range(len(lst)):
                    if lst[j] is inst:
                        idx = j
                        break
                assert idx is not None
                for k, ni in enumerate(nops):
                    lst.insert(idx + k, ni)
    return nc


def _build_bass(reps=1):
    # Bacc (not plain Bass): its finalize() pipeline runs
    # generate_event_semaphores, which splits Tile's multi-wait sync into the
    # 1-wait-per-instruction form this walrus requires.
    nc = bacc.Bacc("TRN2", target_bir_lowering=False)

    # ---- DRAM I/O ----
    x_d = nc.dram_tensor("x", [DIM, NTOK], F32R, kind="ExternalInput")
    # q,k weights, ln_w folded in: (c, m) m in [0, 2048), fp8 x16
    wqk_d = nc.dram_tensor("wqk", [DIM, 2 * DIM], FP8, kind="ExternalInput")
    # v weights: (c, m) m in [0, 1024), fp8 x16
    wv_d = nc.dram_tensor("wv", [DIM, DIM], FP8, kind="ExternalInput")
    # out-proj weights w_out.T: (c, c_out), fp8 x16
    wo_d = nc.dram_tensor("wo", [DIM, DIM], FP8, kind="ExternalInput")
    # rank-2 LN-mean + bias correction planes (rows 0,1 = [a;b]*16, rest 0)
    abqk_d = nc.dram_tensor("abqk8", [128, 2, 2 * DIM], FP8, kind="ExternalInput")
    avbv_d = nc.dram_tensor("avbv8", [128, 2, DIM], FP8, kind="ExternalInput")
    # rope tables, 2 heads stacked (128, 128); f32 copy of sin for psum TT
    cosb_d = nc.dram_tensor("cosb", [128, WIN], BF16, kind="ExternalInput")
    sinf_d = nc.dram_tensor("sinf", [128, WIN], F32, kind="ExternalInput")
    # rotate-half matrix (lhsT), block-diag for 2 heads
    st_d = nc.dram_tensor("st128", [128, 128], BF16, kind="ExternalInput")
    # paired eye-columns for fp8-DR denominator accumulation
    e16_d = nc.dram_tensor("e16", [128, HEADS, HEADS], BF16, kind="ExternalInput")
    sel_d = nc.dram_tensor("sel", [HEADS, KC, 128], BF16, kind="ExternalInput")
    ones8_d = nc.dram_tensor("ones8", [128, 2, 16], FP8, kind="ExternalInput")
    ones_r_d = nc.dram_tensor("ones_r", [128, 1], F32R, kind="ExternalInput")
    ones_row_d = nc.dram_tensor("ones_row", [1, 128], BF16, kind="ExternalInput")
    out_d = nc.dram_tensor("out", [DIM, NTOK], F32, kind="ExternalOutput")

    x_r = x_d.ap().rearrange("(kc p) n -> p kc n", p=128)
    out_r = out_d.ap().rearrange("(kc p) n -> p kc n", p=128)

    with tile.TileContext(nc) as tc:
        with (
            tc.tile_pool(name="wpool", bufs=1) as wpool,
            tc.tile_pool(name="xpool", bufs=CFG["xp_bufs"]) as xpool,
            tc.tile_pool(name="spool", bufs=2) as spool,
            tc.tile_pool(name="qkpool", bufs=CFG["qk_bufs"]) as qkpool,
            tc.tile_pool(name="tpool", bufs=CFG["tp_bufs"]) as tpool,
            tc.tile_pool(name="vpool", bufs=CFG["vp_bufs"]) as vpool,
            tc.tile_pool(name="apool", bufs=CFG["ap_bufs"]) as apool,
            tc.tile_pool(name="ypool", bufs=2) as ypool,
            tc.tile_pool(name="psA", bufs=CFG["psa_bufs"], space="PSUM") as psA,
        ):
            # preload ACT table set 6 (natural_log_exp_and_others): covers
            # Copy/Identity/Square/Exp/Ln, so the finalize fixpoint inserts no
            # per-tile table switches (each costs 1283ns on ACT)
            nc.scalar.add_instruction(
                mybir.InstLoadActFuncSet(
                    name=nc.get_next_instruction_name(), ins=[], outs=[],
                    act_func_set_id=6,
                )
            )
            # ---- resident weights/constants ----
            # (tiles reserved here; const DMAs issued by load_consts() AFTER
            # tile 0's x DMA so ~10us of HWDGE setup doesn't delay it)
            abqk = wpool.tile([128, 2, 2 * DIM], FP8)
            avbv = wpool.tile([128, 2, DIM], FP8)
            cosb = wpool.tile([128, WIN], BF16)
            sinf = wpool.tile([128, WIN], F32)
            st128 = wpool.tile([128, 128], BF16)
            e16 = wpool.tile([128, HEADS, HEADS], BF16)
            sel = wpool.tile([HEADS, KC, 128], BF16)
            ones8 = wpool.tile([128, 2, 16], FP8)
            ones_r = wpool.tile([128, 1], F32R)
            ones_row = wpool.tile([1, 128], BF16)

            def load_consts():
                nc.sync.dma_start(out=ones_r, in_=ones_r_d.ap())
                nc.sync.dma_start(out=ones8, in_=ones8_d.ap())
                nc.sync.dma_start(out=ones_row, in_=ones_row_d.ap())
                nc.sync.dma_start(out=abqk, in_=abqk_d.ap())
                nc.sync.dma_start(out=avbv, in_=avbv_d.ap())
                nc.sync.dma_start(out=cosb, in_=cosb_d.ap())
                nc.sync.dma_start(out=sinf, in_=sinf_d.ap())
                nc.sync.dma_start(out=st128, in_=st_d.ap())
                nc.sync.dma_start(out=e16, in_=e16_d.ap())
                nc.sync.dma_start(out=sel, in_=sel_d.ap())
            eps_t = wpool.tile([1, 1], F32)
            nc.vector.memset(eps_t, EPS)
            zero128 = wpool.tile([128, 1], F32)
            nc.vector.memset(zero128, 0.0)
            # per-tile LN-correction plane (row0 = -mu*rstd, row1 = ones,
            # rest 0; plane1 all 0), double-buffered across tiles
            nm2 = []
            for i in range(2):
                t = wpool.tile([128, 2, NT], FP8, name=f"nm2f8_{i}")
                nc.vector.memset(t, 0.0)
                nc.vector.memset(t[0:2, 0, :], 1.0)
                nm2.append(t)
            # big weight tiles: DMAs issued after tile 0's x load (see below)
            # so the first LN stats aren't queued behind 4MB of weights
            wqk = wpool.tile([128, KC, 2 * DIM], FP8)
            wv = wpool.tile([128, KC, DIM], FP8)
            wo = wpool.tile([128, KC, DIM], FP8)

            def load_big_weights():
                nc.sync.dma_start(out=wqk, in_=wqk_d.ap().rearrange("(kc p) m -> p kc m", p=128))
                nc.sync.dma_start(out=wv, in_=wv_d.ap().rearrange("(kc p) m -> p kc m", p=128))
                nc.sync.dma_start(out=wo, in_=wo_d.ap().rearrange("(kc p) m -> p kc m", p=128))

            def bcast_win(ap_2d, nwin):
                """(128, WIN) tile -> (128, nwin, WIN) stride-0 repeat."""
                return bass.AP(
                    tensor=ap_2d.tensor,
                    offset=ap_2d.offset,
                    ap=[ap_2d.ap[0], [0, nwin], ap_2d.ap[1]],
                )

            def prologue_dma(it):
                tb = it * NT
                _mark(nc, f'ln_stats_{it}')
                # 4 independent tiles: per-chunk consumers unblock as each
                # DMA lands instead of waiting for the full 2MB
                x8s = []
                for j in range(4):
                    t = xpool.tile([128, 2, NT], F32R, tag=f"x8{j}", bufs=3,
                                   name=f"x8_{it}_{j}")
                    nc.sync.dma_start(
                        out=t, in_=x_r[:, 2 * j : 2 * j + 2, tb : tb + NT])
                    x8s.append(t)
                return dict(it=it, tb=tb, x8s=x8s)

            def prologue_compute(holder):
                it, tb, x8s = holder["it"], holder["tb"], holder["x8s"]
                # LN stats: sum(x) via f32r (1 cyc/row at N>=256), sum(x^2)
                # via fp8 squares + DoubleRow pair reduction
                ps_sum = psA.tile([1, NT], F32, tag="mm1", name=f"ps_sum_{it}")
                ps_sq = psA.tile([1, NT], F32, tag="mm1", name=f"ps_sq_{it}")
                for j in range(4):
                    x2 = tpool.tile([128, 2, NT], FP8, tag="tmp",
                                    name=f"x2_{it}_{j}")
                    for i in range(2):
                        kc = 2 * j + i
                        eng = SQ_ENG[kc]
                        if eng == "pool":
                            nc.gpsimd.tensor_mul(out=x2[:, i, :], in0=x8s[j][:, i, :], in1=x8s[j][:, i, :])
                        elif eng == "vec":
                            nc.vector.tensor_mul(out=x2[:, i, :], in0=x8s[j][:, i, :], in1=x8s[j][:, i, :])
                        else:
                            nc.scalar.square(out=x2[:, i, :], in_=x8s[j][:, i, :])
                        nc.tensor.matmul(
                            ps_sum[:, :], ones_r, x8s[j][:, i, :],
                            start=(kc == 0), stop=(kc == KC - 1),
                        )
                    nc.tensor.matmul(
                        ps_sq[:, :], ones8[:, :, 0:1], x2,
                        start=(j == 0), stop=(j == 3),
                        perf_mode=DR,
                    )
                ex = spool.tile([1, NT], F32, tag="sa", name=f"ex_{it}")
                nc.scalar.mul(out=ex, in_=ps_sum[:, :], mul=1.0 / DIM)
                nmu2 = spool.tile([1, NT], F32, tag="sc", name=f"nmu2_{it}")
                nc.vector.scalar_tensor_tensor(
                    out=nmu2, in0=ex, scalar=-1.0, in1=ex,
                    op0=mybir.AluOpType.mult, op1=mybir.AluOpType.mult,
                )
                var = spool.tile([1, NT], F32, tag="sb", name=f"var_{it}")
                nc.vector.scalar_tensor_tensor(
                    out=var, in0=ps_sq[:, :], scalar=1.0 / DIM, in1=nmu2,
                    op0=mybir.AluOpType.mult, op1=mybir.AluOpType.add,
                )
                # rstd = 1/sqrt(var+eps) as exp(-0.5*ln(var+eps)): Ln and Exp
                # share one ACT table set (Sqrt does not, and a per-tile table
                # switch costs 1283ns)
                lnv = spool.tile([1, NT], F32, tag="sc", name=f"lnv_{it}")
                nc.scalar.activation(
                    out=lnv, in_=var, func=mybir.ActivationFunctionType.Ln,
                    bias=eps_t[:, :], scale=1.0,
                )
                rstd_bf = spool.tile([1, NT], BF16, tag="sbf", bufs=4,
                                     name=f"rstd_bf_{it}")
                nc.scalar.activation(
                    out=rstd_bf, in_=lnv, func=mybir.ActivationFunctionType.Exp,
                    bias=zero128[0:1, :], scale=-0.5,
                )
                # write row0 (= -mu*rstd) of this tile's correction plane
                nmr1 = nm2[it % 2]
                nc.vector.scalar_tensor_tensor(
                    out=nmr1[0:1, 0, :], in0=ex, scalar=-1.0, in1=rstd_bf,
                    op0=mybir.AluOpType.mult, op1=mybir.AluOpType.mult,
                )
                holder.update(nmr1=nmr1, rstd_bf=rstd_bf)

            def make_xs(holder):
                """Emitted late (with qk prep): by then rstd_bf is done, so
                the rb broadcast MM doesn't park the in-order PE queue."""
                it, x8s = holder["it"], holder["x8s"]
                rstd_bf = holder["rstd_bf"]
                # broadcast rstd to all partitions via a K=1 rank-1 matmul
                # (ones column x rstd row) into PSUM; x_s reads it there.
                rb_ps = psA.tile([128, NT], F32, tag="mm1", name=f"rb_ps_{it}")
                nc.tensor.matmul(rb_ps[:, :], ones_row, rstd_bf,
                                 start=True, stop=True)
                # x_s = x * rstd in fp8 (mean handled by rank-1 matmul downstream)
                x_s = xpool.tile([128, KC, NT], FP8, tag="xs", name=f"x_s_{it}")
                rb_b = bass.AP(tensor=rb_ps.tensor, offset=rb_ps.offset,
                               ap=[rb_ps.ap[0], [0, 2], rb_ps.ap[1]])
                nc.vector.tensor_mul(out=x_s[:, 0:2, :], in0=x8s[0], in1=rb_b)
                nc.vector.tensor_mul(out=x_s[:, 2:4, :], in0=x8s[1], in1=rb_b)
                nc.vector.tensor_mul(out=x_s[:, 4:6, :], in0=x8s[2], in1=rb_b)
                # Pool cannot read PSUM: bounce rstd row through SBUF for it
                rb_sb = spool.tile([128, NT], BF16, tag="rbsb", name=f"rb_sb_{it}")
                nc.scalar.copy(out=rb_sb, in_=rb_ps[:, :])
                rb_b3 = bass.AP(tensor=rb_sb.tensor, offset=rb_sb.offset,
                                ap=[rb_sb.ap[0], [0, 2], rb_sb.ap[1]])
                nc.gpsimd.tensor_mul(out=x_s[:, 6:8, :], in0=x8s[3], in1=rb_b3)
                holder["x_s"] = x_s

            def qk_pair(st, mc0):
                it, x_s, nmr1, qk = st["it"], st["x_s"], st["nmr1"], st["qk"]
                if mc0 == 0:
                    _mark(nc, f'qkrope_{it}')
                for mc in (mc0, mc0 + 1):
                    ps_qk = psA.tile([128, NT], F32, tag="mm1",
                                     name=f"ps_qk_{it}_{mc}")
                    for k2 in range(KC // 2):
                        nc.tensor.matmul(
                            ps_qk[:, :],
                            wqk[:, 2 * k2 : 2 * k2 + 2, mc * 128 : (mc + 1) * 128],
                            x_s[:, 2 * k2 : 2 * k2 + 2, :],
                            start=(k2 == 0), stop=False,
                            perf_mode=DR,
                        )
                    nc.tensor.matmul(
                        ps_qk[:, :],
                        abqk[:, :, mc * 128 : (mc + 1) * 128],
                        nmr1,
                        start=False, stop=True,
                        perf_mode=DR,
                    )
                    nc.scalar.mul(
                        out=qk[:, mc, :], in_=ps_qk[:, :], mul=1.0 / WSCALE,
                    )
                    ps_u = psA.tile([128, NT], F32, tag="mm1",
                                    name=f"ps_u_{it}_{mc}")
                    nc.tensor.matmul(ps_u[:, :], st128, qk[:, mc, :],
                                     start=True, stop=True)
                    t2 = tpool.tile([128, NT], BF16, tag="tmp",
                                    name=f"t2_{it}_{mc}")
                    nc.vector.tensor_mul(
                        out=t2, in0=ps_u[:, :], in1=bcast_win(sinf, WPT))
                    tmp = tpool.tile([128, NT], BF16, tag="tmp",
                                     name=f"tmp_{it}_{mc}")
                    nc.gpsimd.tensor_mul(
                        out=tmp, in0=qk[:, mc, :], in1=bcast_win(cosb, WPT))
                    nc.vector.tensor_add(out=qk[:, mc, :], in0=tmp, in1=t2)

            def vt_sub(st, sub):
                it, x_s, nmr1, vt = st["it"], st["x_s"], st["nmr1"], st["vt"]
                if sub == 0:
                    _mark(nc, f'vt_{it}')
                for nh in range(2):
                    ncol = slice(nh * 512, (nh + 1) * 512)
                    ps_vt = psA.tile([128, 512], F32, tag="mm1",
                                     name=f"ps_vt_{it}_{sub}_{nh}")
                    for k2 in range(KC // 2):
                        nc.tensor.matmul(
                            ps_vt[:, :],
                            x_s[:, 2 * k2 : 2 * k2 + 2, sub * 128 : (sub + 1) * 128],
                            wv[:, 2 * k2 : 2 * k2 + 2, ncol],
                            start=(k2 == 0), stop=False,
                            perf_mode=DR,
                        )
                    nc.tensor.matmul(
                        ps_vt[:, :],
                        nmr1[:, :, sub * 128 : (sub + 1) * 128],
                        avbv[:, :, ncol],
                        start=False, stop=True,
                        perf_mode=DR,
                    )
                    nc.scalar.mul(out=vt[:, sub, ncol], in_=ps_vt[:, :],
                                  mul=1.0 / WSCALE)

            def ecol(hh):
                return (hh % 2) * 512 + (hh // 2) * WIN

            def attn_scores(st, wl):
                it, qk = st["it"], st["qk"]
                if wl == 0:
                    _mark(nc, f'attn_{it}')
                    st["attn_t"] = apool.tile(
                        [128, KC, NT], FP8, tag="attn", bufs=2,
                        name=f"attn_t_{it}")
                wslc = slice(wl * WIN, (wl + 1) * WIN)
                # parity-split so matmuls with different contraction
                # row-groups (operand base partition 0 vs 64) never share
                # a PSUM bank (HW faults otherwise).
                expt = apool.tile([128, 2, 8 * WIN], BF16, tag="expt", bufs=3,
                                  name=f"expt_{it}_{wl}")
                st[f"expt{wl}"] = expt
                for hg in range(2):
                    for par in range(2):
                        ps_sc = psA.tile([128, 4 * WIN], F32, tag="mm1",
                                         name=f"ps_sc_{it}_{wl}_{hg}_{par}")
                        po = par * 64
                        for j in range(4):
                            h = hg * 8 + 2 * j + par
                            qh = qk[po : po + 64, h // 2, wslc]
                            kh = qk[po : po + 64, 8 + h // 2, wslc]
                            nc.tensor.matmul(
                                ps_sc[:, j * WIN : (j + 1) * WIN],
                                kh, qh, start=True, stop=True,
                            )
                        nc.scalar.activation(
                            out=expt[:, hg, par * 512 : (par + 1) * 512],
                            in_=ps_sc[:, :],
                            func=mybir.ActivationFunctionType.Exp,
                            bias=zero128[:, :], scale=0.125,
                        )

            def attn_denom_av(st, wl):
                it, vt = st["it"], st["vt"]
                expt = st[f"expt{wl}"]
                ps_d = psA.tile([HEADS, WIN], F32, tag="mm1",
                                name=f"ps_d_{it}_{wl}")
                for hg in range(2):
                    for hh in range(8):
                        h = hg * 8 + hh
                        nc.tensor.matmul(
                            ps_d[:, :], e16[:, h, :],
                            expt[:, hg, ecol(hh) : ecol(hh) + WIN],
                            start=(h == 0), stop=(h == HEADS - 1),
                        )
                rd = spool.tile([HEADS, WIN], F32, tag="rd",
                                name=f"rd_{it}_{wl}")
                nc.vector.reciprocal_approx_fast(out=rd, in_=ps_d[:, :])
                rd_bf = spool.tile([HEADS, WIN], BF16, tag="rd",
                                   name=f"rd_bf_{it}_{wl}")
                nc.scalar.copy(out=rd_bf, in_=rd)
                st[f"rdbf{wl}"] = rd_bf
                ps_ats = []
                for hg in range(2):
                    ps_at = psA.tile([128, 4 * WIN], F32, tag="mm1",
                                     name=f"ps_at_{it}_{wl}_{hg}")
                    for hh in range(8):
                        h = hg * 8 + hh
                        po = (h % 2) * 64
                        c = (h // 2) % 4
                        nc.tensor.matmul(
                            ps_at[po : po + 64, c * WIN : (c + 1) * WIN],
                            vt[:, wl, h * 64 : (h + 1) * 64],
                            expt[:, hg, ecol(hh) : ecol(hh) + WIN],
                            start=True, stop=True,
                            tile_position=(0, po),
                        )
                    ps_ats.append(ps_at)
                st[f"psat{wl}"] = ps_ats

            def attn_norm(st, wl):
                it = st["it"]
                attn_t = st["attn_t"]
                rd_bf = st[f"rdbf{wl}"]
                ps_ats = st[f"psat{wl}"]
                wslc = slice(wl * WIN, (wl + 1) * WIN)
                for hg in range(2):
                    ps_bc = psA.tile([128, 4 * WIN], F32, tag="mm1",
                                     name=f"ps_bc_{it}_{wl}_{hg}")
                    for cc in range(4):
                        c = hg * 4 + cc
                        nc.tensor.matmul(
                            ps_bc[:, cc * WIN : (cc + 1) * WIN],
                            sel[:, c, :], rd_bf, start=True, stop=True,
                        )
                    # drain to SBUF: DVE stt below may read only one PSUM
                    bcw = apool.tile([128, 4, WIN], BF16, tag="bc",
                                     name=f"bcw_{it}_{wl}_{hg}")
                    nc.scalar.copy(
                        out=bcw, in_=ps_bc.rearrange("p (c i) -> p c i", c=4))
                    # fused psum drain + 1/denom mul + x8 fp8 range scale
                    nc.vector.scalar_tensor_tensor(
                        out=attn_t[:, 4 * hg : 4 * hg + 4, wslc],
                        in0=ps_ats[hg].rearrange("p (c i) -> p c i", c=4),
                        scalar=8.0,
                        in1=bcw,
                        op0=mybir.AluOpType.mult, op1=mybir.AluOpType.mult,
                    )

            def outproj_batch(st, mcs):
                it, tb, attn_t = st["it"], st["tb"], st["attn_t"]
                x8s = st["x8s"]
                if mcs[0] == 0:
                    _mark(nc, f'outproj_{it}')
                # window-major MM order: all wl=0 MMs (ready earliest) first,
                # so the in-order PE queue never parks on a late window
                pys = {}
                for mc in mcs:
                    pys[mc] = psA.tile([128, NT], F32, tag="mm1",
                                       name=f"ps_y_{it}_{mc}")
                for wl in range(WPT):
                    wslc = slice(wl * WIN, (wl + 1) * WIN)
                    for mc in mcs:
                        for k2 in range(KC // 2):
                            nc.tensor.matmul(
                                pys[mc][:, wslc],
                                wo[:, 2 * k2 : 2 * k2 + 2, mc * 128 : (mc + 1) * 128],
                                attn_t[:, 2 * k2 : 2 * k2 + 2, wslc],
                                start=(wl == 0 and k2 == 0),
                                stop=(wl == WPT - 1 and k2 == KC // 2 - 1),
                                perf_mode=DR,
                            )
                for mc in mcs:
                    y = ypool.tile([128, NT], F32, tag="y", name=f"y_{it}_{mc}")
                    nc.vector.scalar_tensor_tensor(
                        out=y, in0=pys[mc][:, :], scalar=1.0 / (WSCALE * 8.0),
                        in1=x8s[mc // 2][:, mc % 2, :],
                        op0=mybir.AluOpType.mult, op1=mybir.AluOpType.add,
                    )
                    nc.sync.dma_start(
                        out=out_r[:, mc, tb : tb + NT], in_=y,
                    )

            def start_tile(it):
                holder = prologue_dma(it)
                holder["qk"] = qkpool.tile([128, 16, NT], BF16, tag="qk",
                                           name=f"qk_{it}")
                holder["vt"] = vpool.tile([128, WPT, DIM], BF16, tag="vt",
                                          name=f"vt_{it}")
                return holder

            def produce_steps(holder):
                """Tile prep: LN compute, 8 qk pairs, 4 vt subs."""
                steps = [lambda: prologue_compute(holder)]
                steps += [lambda g=g: qk_pair(holder, 2 * g) for g in range(8)]
                steps += [lambda s=s: vt_sub(holder, s) for s in range(WPT)]
                return steps

            its = [t for _ in range(reps) for t in range(NTILES)]
            states = {0: start_tile(its[0])}
            load_consts()
            load_big_weights()
            _p0 = produce_steps(states[0])
            _p0[0]()
            make_xs(states[0])
            for f in _p0[1:]:
                f()
            if len(its) > 1:
                states[1] = start_tile(its[1])
            for idx, it in enumerate(its):
                st = states.pop(idx)
                # x DMA for tile idx+2 (two rounds ahead; its buffer frees
                # when outproj(idx-1) finishes reading)
                if idx + 2 < len(its):
                    states[idx + 2] = start_tile(its[idx + 2])
                # tile idx+1 prep steps: LN compute first (its x landed last
                # round), qk/vt spliced into outproj below
                nxt = states.get(idx + 1)
                if nxt is not None:
                    prod = produce_steps(nxt)
                    prod[0]()
                    if CFG["xs_at"] < 0:
                        make_xs(nxt)
                    rest = prod[1:]
                else:
                    rest = []
                # attention software pipeline: scores run 2+ windows ahead of
                # denominator+attnV, which run ahead of the normalize step
                sc = lambda w: attn_scores(st, w)
                dat = lambda w: attn_denom_av(st, w)
                bc = lambda w: attn_norm(st, w)
                qk_in = dict(CFG["qk_in_attn"])
                for n_step, i_f in enumerate(CFG["attn_order"]):
                    kind, w = i_f
                    (sc if kind == 's' else dat if kind == 'd' else bc)(w)
                    if n_step == CFG["xs_at"] and nxt is not None:
                        make_xs(nxt)
                    if n_step in qk_in and rest:
                        k = qk_in[n_step]
                        if k < len(rest) and rest[k] is not None:
                            rest[k]()
                            rest[k] = None
                # outproj interleaved with next tile's qk/vt prep; last
                # round has nothing to interleave -> one window-major batch
                nb = CFG["op_batches"]
                msz = KC // nb
                mid = [
                    (lambda b=b: outproj_batch(st, list(range(b * msz, (b + 1) * msz))))
                    for b in range(nb)
                ]
                seq = []
                ri = 0
                for m in mid:
                    if ri < len(rest):
                        seq.append(rest[ri])
                        ri += 1
                    seq.append(m)
                seq += rest[ri:]
                for f in seq:
                    if f is not None:
                        f()
    nc.finalize()
    return nc


def _host_prep(x, ln_w, ln_b, w_qkv, w_out):
    """Shared (non-x) device inputs, host-precomputed."""
    bf = ml_dtypes.bfloat16
    wqkv_s = (w_qkv * ln_w[None, :]).astype(np.float32)  # (3C, C) scaled
    wT = np.ascontiguousarray(wqkv_s.T)  # (C, 3C)
    b_qkv = (w_qkv @ ln_b).astype(np.float32)  # (3C,)
    a_qkv = wqkv_s.sum(axis=1).astype(np.float32)  # (3C,)

    f8 = ml_dtypes.float8_e4m3
    ins = {}
    ins["wqk"] = (np.ascontiguousarray(wT[:, : 2 * DIM]) * WSCALE).astype(f8)
    ins["wv"] = (np.ascontiguousarray(wT[:, 2 * DIM :]) * WSCALE).astype(f8)
    ins["wo"] = (np.ascontiguousarray(w_out.T) * WSCALE).astype(f8)
    # rank-2 correction planes: rows 0,1 = [a;b]*16, rest zero; plane1 zero
    abqk = np.zeros((128, 2, 2 * DIM), np.float32)
    abqk[0, 0] = a_qkv[: 2 * DIM] * WSCALE
    abqk[1, 0] = b_qkv[: 2 * DIM] * WSCALE
    ins["abqk8"] = abqk.astype(f8)
    avbv = np.zeros((128, 2, DIM), np.float32)
    avbv[0, 0] = a_qkv[2 * DIM :] * WSCALE
    avbv[1, 0] = b_qkv[2 * DIM :] * WSCALE
    ins["avbv8"] = avbv.astype(f8)

    inv_freq = 1.0 / 10000 ** (np.arange(0, DHEAD, 2, dtype=np.float32) / DHEAD)
    pos = np.arange(WIN, dtype=np.float32)
    freqs = np.concatenate([np.outer(pos, inv_freq)] * 2, axis=-1)  # (WIN, 64)
    cos_t = np.cos(freqs).T.astype(np.float32)  # (64, WIN)
    sin_t = np.sin(freqs).T.astype(np.float32)
    ins["cosb"] = np.tile(cos_t, (2, 1)).astype(bf)
    ins["sinf"] = np.tile(sin_t, (2, 1)).astype(np.float32)

    S = np.zeros((DHEAD, DHEAD), np.float32)
    S[: DHEAD // 2, DHEAD // 2 :] = -np.eye(DHEAD // 2)
    S[DHEAD // 2 :, : DHEAD // 2] = np.eye(DHEAD // 2)
    ST = S.T
    st128 = np.zeros((128, 128), np.float32)
    st128[:64, :64] = ST
    st128[64:, 64:] = ST
    ins["st128"] = st128.astype(bf)

    e = np.zeros((128, HEADS, HEADS), np.float32)
    for h in range(HEADS):
        e[:, h, h] = 1.0
    ins["e16"] = e.astype(bf)

    # 1/denom broadcast selector: ps_bc[p, cg, j] = rd[head(cg, p), j]
    selm = np.zeros((HEADS, KC, 128), np.float32)
    for cg in range(KC):
        hg, cl = cg // 4, cg % 4
        selm[8 * hg + 2 * cl, cg, :64] = 1.0
        selm[8 * hg + 2 * cl + 1, cg, 64:] = 1.0
    ins["sel"] = selm.astype(bf)


    ins["ones8"] = np.pad(np.ones((128, 2, 1), np.float32), ((0,0),(0,0),(0,15))).astype(f8)
    ins["ones_r"] = np.ones((128, 1), np.float32)
    ins["ones_row"] = np.ones((1, 128), np.float32).astype(bf)
    return ins


def kernel(x, ln_w, ln_b, w_qkv, w_out, _want_trace=False):
    x = np.asarray(x, dtype=np.float32)
    shared = _host_prep(
        np.asarray(x, np.float32),
        np.asarray(ln_w, np.float32),
        np.asarray(ln_b, np.float32),
        np.asarray(w_qkv, np.float32),
        np.asarray(w_out, np.float32),
    )

    if "nc" not in _CACHED:
        _CACHED["nc"] = _build_bass()
    nc = _CACHED["nc"]

    in_maps = []
    for core in range(NCORE):
        b, half = core // 2, core % 2
        xs = np.ascontiguousarray(x[b, :, half * NTOK : (half + 1) * NTOK])
        m = dict(shared)
        m["x"] = xs
        in_maps.append(m)

    res = run_bass_kernel_spmd(
        nc, in_maps, core_ids=list(range(NCORE)), trace=_want_trace
    )
    out = np.empty((B, DIM, T), np.float32)
    for core in range(NCORE):
        b, half = core // 2, core % 2
        out[b, :, half * NTOK : (half + 1) * NTOK] = res.results[core]["out"]
    if _want_trace:
        _CACHED["last_trace"] = res
    return out

